# revision 62
# baseline (speedup 1.0000x reference)
"""Trainium2 Bass kernel for dense multi-head self-attention.

Reference computation (fp32):
    xn  = rms_norm(x) * (gamma + 1)          # F.normalize(x) * sqrt(D) * (gamma+1)
    qkv = xn @ w_qkv ; split into q, k, v    # heads H=16, dim_head 64
    out = softmax(q k^T / sqrt(64)) v
    y   = out @ w_out
Sharding (8 cores): data-parallel over batch (2), tensor-parallel over heads
(16 -> 4 groups of 4). Core c handles batch c//4, head group c%4. w_qkv is
column-sliced, w_out row-sliced per head group; each core emits a partial
[2048, 1024] output which the host sums per batch. No cross-device
communication inside the kernel.

v3 design notes (per-core, all bf16 matmuls, fp32 PSUM accumulation):
  - gamma+1 and the 1/sqrt(64) q-scale are folded into w_qkv on the HOST
    (fp64), and the rms scale rs[t] = sqrt(d)/||x_t|| (0.02% of the FLOPs)
    is computed on the host and shipped as a [n] f32 input. This removes
    every ACT function except Exp -> exactly one ACT_TABLE_LOAD, which
    matters because Ln/Sqrt/Reciprocal live in different table sets and
    each switch costs ~2.7us of serialized ACT time.
  - softmax exp of S^T chunks is SPLIT between ACT (true Exp) and DVE
    (Schraudolph fast-exp): bits = round(s*128/ln2 + (16256-5.51)) as
    int16, bitcast to bf16 => exp(s)*(1 +- 3%). One tensor_scalar op per
    [128,1024] chunk, psum->sbuf. The split fraction balances ACT vs DVE
    occupancy so the PE attention matmuls stay the critical path.
  - attention-out psum tiles are drained to SBUF by ACT copies (no table,
    psum-adjacent engine) as soon as each head-pair finishes, so the psum
    slots never wait on the softmax normalization chain (the v2 stall that
    caused ~3.5us PE gaps per head-pair and HAM clock-gate oscillation).
  - softmax denominators (ones-column of the V matmul) are ACT-copied onto
    4 separate partitions of a [4,512] pack per query group; ONE DVE
    reciprocal per qg costs 4.3us (DVE reciprocal is 8 cyc per FREE
    element, partition-parallel -- row-by-row it was 66us in the baseline).
  - normalization multiplies (attn_out * 1/denom broadcast) run on GPSIMD,
    which is otherwise idle; DVE is loaded with its exp share.
  - scores S^T per head-pair use tile_position row packing (dh=64), with
    the two heads' matmuls interleaved so consecutive PE instructions hit
    different row groups and run concurrently.
"""

import numpy as np

import concourse.bass as bass  # noqa: F401
import concourse.mybir as mybir
import concourse.tile as tile
from concourse import bacc
from concourse.bass_utils import run_bass_kernel_spmd

# Problem constants (hardcoded per contract; kernel.py must be self-contained).
B = 2          # batch
N = 2048       # sequence length
D = 1024       # model dim
H = 16         # total heads
DH = 64        # dim per head
HL = 4         # heads per core
DQ = HL * DH   # 256 = per-core q/k/v width
NCORES = 8

P = 128        # partitions

F32 = mybir.dt.float32
BF16 = mybir.dt.bfloat16
I16 = mybir.dt.int16

# Schraudolph fast-exp constants in bf16-exponent space:
#   bits = s * (2^7 / ln 2) + (127*2^7 - c);  bitcast<bf16>(bits) ~ exp(s)
# c = 2^7 * 0.043 balances the (1+f)/2^f linear-interp error to +-3%.
EXP_A = 128.0 / np.log(2.0)
EXP_B = 16256.0 - 5.513


def build_attention_kernel_v3(n=N, d=D, hl=HL, dh=DH, dve_sixteenths=7,
                              ov_delay=8):
    """Build the single-core SPMD Bass program (v3, all-bf16).

    dve_sixteenths: of the 16 (kcp, sub) exp units per (qg, hp), how many
    route to the DVE fast-exp instead of ACT Exp.
    """
    PDT = BF16
    ADT = BF16
    dq = hl * dh
    ndc = d // P        # dim chunks of 128
    nt4 = n // 512      # token tiles of 512
    nt16 = n // P       # token tiles of 128
    kc_n = n // P       # key chunks of 128
    qg_n = n // 512     # query groups of 512
    hp_n = hl // 2      # head pairs

    # Routing of the 16 exp units (kcp 0..7 x sub 0..1) to DVE: strict
    # sub-alternation (sub0 -> ACT, sub1 -> DVE) so every kcp's exp wall is
    # max(570, 659) < the PE time per kcp -- except kcp5, whose DVE unit
    # moves to ACT; the per-qg DVE reciprocal is scheduled exactly there so
    # it never delays a fast-exp the scores are waiting on.
    dve_unit = [(u % 2 == 1) and u != 11 for u in range(16)]

    nc = bacc.Bacc()
    # xT arrives already rms-normalized (host folds rs[t] = sqrt(d)/||x_t||
    # into the columns), so the q/k/v psum drains are plain copies. All
    # inputs are HOST-PRE-TILED to the on-chip layout so every DMA reads
    # long contiguous runs (the naive row-strided patterns cost ~10ns per
    # descriptor -- the wqkv load alone was 1024 descriptors = 10us on the
    # first-matmul critical path).
    n_halves_ = 4 if n >= 2048 else (2 if n >= 1024 else 1)
    xT_d = nc.declare_dram_parameter(
        "xT", [n_halves_, P, d // P, n // n_halves_], PDT, isOutput=False
    )
    wqkv_d = nc.declare_dram_parameter(
        "wqkv", [P, d // P, 3 * dq], PDT, isOutput=False
    )
    wout_d = nc.declare_dram_parameter(
        "wout", [P, dq // P, d], PDT, isOutput=False
    )
    # Partial outputs in bf16 (host sums the head groups in f32): halves
    # the store traffic; quantization is ~0.2% of the summed output.
    out_d = nc.declare_dram_parameter("out", [n, d], BF16, isOutput=True)

    kc2_n = dq // P     # contraction chunks for the output projection
    on_n = d // 512     # output-column tiles
    n_halves = 4 if n >= 2048 else (2 if n >= 1024 else 1)
    nh = n // n_halves

    with tile.TileContext(nc) as tc:
        with (
            # 2 KiB/partition slots for the expS^T tiles.
            tc.tile_pool(name="big", bufs=16) as big,
            tc.tile_pool(name="consts", bufs=1) as consts,
            tc.tile_pool(name="weights", bufs=1) as weights,
            tc.tile_pool(name="qkt", bufs=1) as qkt,
            tc.tile_pool(name="vpool", bufs=1) as vpool,
            tc.tile_pool(name="otc", bufs=2) as otc_pool,
            tc.tile_pool(name="recip", bufs=2) as recip,
            tc.tile_pool(name="aot", bufs=2) as aot_pool,
            tc.tile_pool(name="outsb", bufs=3) as outsb,
            tc.tile_pool(name="st_ps", bufs=4, space="PSUM") as st_ps,
            tc.tile_pool(name="ot_ps", bufs=2, space="PSUM") as ot_ps,
            tc.tile_pool(name="proj_ps", bufs=2, space="PSUM") as proj_ps,
        ):
            # wqkv on the ACT hwdge queue so the x tiles stream on the SP
            # queue concurrently from t=0.
            # wqkv: q|k columns first (they gate the first matmul), the v
            # columns behind the first x half on the other queue.
            wqkv_sb = weights.tile([P, ndc, 3 * dq], PDT, tag="wqkv")
            nc.scalar.dma_start(
                out=wqkv_sb[:, :, 0 : 2 * dq], in_=wqkv_d[:, :, 0 : 2 * dq]
            )
            xbig = consts.tile([P, ndc, n], PDT, tag="xbig")
            for h2 in range(n_halves):
                for dcg in range(2):  # split each half over the two queues
                    eng = nc.sync if (h2 + dcg) % 2 == 0 else nc.scalar
                    eng.dma_start(
                        out=xbig[:, dcg * ndc // 2 : (dcg + 1) * ndc // 2,
                                 h2 * nh : (h2 + 1) * nh],
                        in_=xT_d[h2, :, dcg * ndc // 2 : (dcg + 1) * ndc // 2],
                    )
                if h2 == 0:
                    nc.scalar.dma_start(
                        out=wqkv_sb[:, :, 2 * dq : 3 * dq],
                        in_=wqkv_d[:, :, 2 * dq : 3 * dq],
                    )
            # Late load: only needed by the output projection.
            wout_sb = weights.tile([P, kc2_n, d], PDT, tag="wout")
            nc.scalar.dma_start(out=wout_sb, in_=wout_d[:])

            def xt_slice(dc, lo, size):
                return xbig[:, dc, lo : lo + size]

            ones_bf = consts.tile([P, nt16 * hl], PDT, tag="ones_bf")
            nc.vector.memset(ones_bf, 1.0)

            # q^T / k^T projections: [128 rows = head-pair x 64 dims, tokens].
            # rms normalization (rs per token) applied at the psum drain.
            qT = qkt.tile([P, hp_n, n], ADT, tag="qT")
            kT = qkt.tile([P, hp_n, n], ADT, tag="kT")
            for h2 in range(n_halves):
                for hp in range(hp_n):
                    for part in range(2):  # 0 = q, 1 = k
                        for nt in range(h2 * nt4 // n_halves, (h2 + 1) * nt4 // n_halves):
                            ps = proj_ps.tile([P, 512], F32, tag="proj", name="psqk")
                            off = part * dq + hp * P
                            for dc in range(ndc):
                                nc.tensor.matmul(
                                    ps,
                                    lhsT=wqkv_sb[:, dc, off : off + P],
                                    rhs=xt_slice(dc, nt * 512, 512),
                                    start=(dc == 0),
                                    stop=(dc == ndc - 1),
                                )
                            dst = qT if part == 0 else kT
                            nc.vector.tensor_copy(
                                dst[:, hp, nt * 512 : (nt + 1) * 512], ps
                            )

            # v projection in natural orientation [token, head*dh], with a
            # ones column appended per head (softmax denominator trick).
            # The 16 v tiles are emitted interleaved into the first query
            # group's score stream (the OV pipeline consumes v_sb only
            # ov_delay chunks behind), so the exp engines start earlier.
            v_sb = vpool.tile([P, nt16, hl, dh + 1], ADT, tag="v")
            nc.vector.tensor_copy(
                v_sb[:, :, :, dh : dh + 1].rearrange("p a b o -> p (a b o)"),
                ones_bf,
            )

            def emit_v(ntt):
                ps = proj_ps.tile([P, dq], F32, tag="proj", name="psv")
                for dc in range(ndc):
                    nc.tensor.matmul(
                        ps,
                        lhsT=xt_slice(dc, ntt * P, P),
                        rhs=wqkv_sb[:, dc, 2 * dq : 3 * dq],
                        start=(dc == 0),
                        stop=(dc == ndc - 1),
                    )
                nc.vector.tensor_copy(
                    v_sb[:, ntt, :, 0:dh],
                    ps.rearrange("p (h dd) -> p h dd", h=hl),
                )

            # Attention + output projection, one query group (512) at a
            # time, software-pipelined across engines:
            #   PE:     scores (row-packed head pair) -> OV (lagged ov_delay)
            #   ACT:    Exp of (1 - alpha) of the S^T chunks; psum->sbuf
            #           drains of finished attention-out tiles
            #   DVE:    fast-exp of alpha of the chunks; denominator recip
            #   GPSIMD: 1/denom broadcast + normalize multiply into aot
            out_ap = out_d[:]
            pending_otcopy = []
            pending_norm = []
            pending_outproj = []

            def emit_otcopy(qg, hp, ots, otc, dpk):
                # Drain attention-out psum [65,512] per sub into the otc
                # staging tile (one ACT copy; the denominator row 64 rides
                # along and is picked up by the norm DMA from otc[64]).
                for sub in range(2):
                    u = hp * 2 + sub
                    nc.scalar.copy(otc[:, u, :], ots[sub][0 : dh + 1, :])

            def emit_norm(qg, otc, dpk, aot, hps=(0, 1)):
                # Denominator rows sit side by side on partition 64 of otc;
                # SBUF->SBUF DMA spreads them over separate partitions so
                # ONE DVE reciprocal covers them lane-parallel (reciprocal
                # costs 8 cyc per FREE element), then a DMA brings the
                # results back to partition 0 for the gpsimd broadcasts.
                us = [hp * 2 + s for hp in hps for s in range(2)]
                nu = len(us)
                dpn = recip.tile([nu, 512], F32, tag="dp4", name=f"dp{qg}_{us[0]}")
                nc.gpsimd.dma_start(
                    out=dpn,
                    in_=otc[dh : dh + 1, us[0] : us[0] + nu, :].rearrange(
                        "o u t -> o (u t)"
                    ),
                )
                rrn = recip.tile([nu, 512], F32, tag="rr4", name=f"rr{qg}_{us[0]}")
                nc.vector.reciprocal(rrn, dpn)
                rrow = recip.tile([1, nu, 512], F32, tag="rrow", name=f"rw{qg}_{us[0]}")
                nc.gpsimd.dma_start(
                    out=rrow[0:1].rearrange("o u t -> o (u t)"),
                    in_=rrn,
                )
                for i, u in enumerate(us):
                    hp, sub = u // 2, u % 2
                    rb = recip.tile([dh, 512], F32, tag="rbcast", name="rb")
                    nc.gpsimd.partition_broadcast(rb, rrow[0:1, i, :], channels=dh)
                    nc.vector.tensor_mul(
                        out=aot[sub * dh : (sub + 1) * dh, hp, :],
                        in0=otc[0:dh, u, :],
                        in1=rb,
                    )

            def emit_outproj_j(qg, aot, j):
                # One 128-token row block: kc2-major matmul order reuses
                # each stationary for both output-column tiles, halving the
                # weight loads; drains alternate DVE/ACT to avoid bursts.
                ntt = qg * 4 + j
                pss = [
                    proj_ps.tile([P, 512], F32, tag="proj", name="pso")
                    for _ in range(on_n)
                ]
                for kc2 in range(kc2_n):
                    for on in range(on_n):
                        nc.tensor.matmul(
                            pss[on],
                            lhsT=aot[:, kc2, j * P : (j + 1) * P],
                            rhs=wout_sb[:, kc2, on * 512 : (on + 1) * 512],
                            start=(kc2 == 0),
                            stop=(kc2 == kc2_n - 1),
                        )
                for on in range(on_n):
                    ob = outsb.tile([P, 512], BF16, tag="outsb", name="ob")
                    if on % 2 == 0:
                        nc.vector.tensor_copy(ob, pss[on])
                    else:
                        nc.scalar.copy(ob, pss[on])
                    eng = nc.sync if (j + on) % 2 == 0 else nc.scalar
                    eng.dma_start(
                        out=out_ap[ntt * P : (ntt + 1) * P, on * 512 : (on + 1) * 512],
                        in_=ob,
                    )

            def emit_outproj(qg, aot):
                for j in range(4):
                    emit_outproj_j(qg, aot, j)

            # The OV queue carries across head-pair and query-group
            # boundaries: while the tail OVs of one block wait on their exp
            # results, the next block's score matmuls keep the PE busy.
            ov_q = []

            def do_ov(ctx, kc, ests, half):
                qg, hp, ots, otc, dpk, aot = ctx
                for sub in range(2):
                    nc.tensor.matmul(
                        ots[sub],
                        lhsT=v_sb[:, kc, hp * 2 + sub, :],
                        rhs=ests[sub][:, half * 512 : (half + 1) * 512],
                        start=(kc == 0),
                        stop=(kc == kc_n - 1),
                    )
                if kc == kc_n - 1:
                    # Head pair finished: free the psum slots immediately
                    # (ACT copies), queue normalization work per qg. The
                    # LAST qg normalizes per head-pair and projects inline
                    # so the end-of-kernel flush chain is short.
                    emit_otcopy(qg, hp, ots, otc, dpk)
                    if qg == qg_n - 1:
                        emit_norm(qg, otc, dpk, aot, hps=(hp,))
                        if hp == hp_n - 1:
                            emit_outproj(qg, aot)
                    elif hp == hp_n - 1:
                        pending_norm.append((qg, otc, dpk, aot))
                        pending_outproj.append((qg, aot, 0))

            for qg in range(qg_n):
                qs = slice(qg * 512, (qg + 1) * 512)
                aot = aot_pool.tile([P, kc2_n, 512], PDT, tag="aot", name=f"aot{qg}")
                otc = otc_pool.tile([dh + 1, 4, 512], F32, tag="otc", name=f"otc{qg}")
                dpk = None
                for hp in range(hp_n):
                    ots = [
                        ot_ps.tile([dh + 1, 512], F32, tag="ot", name=f"ot{qg}_{hp}_{s}")
                        for s in range(2)
                    ]
                    ctx = (qg, hp, ots, otc, dpk, aot)
                    for kcp in range(kc_n // 2):
                        if qg == 0 and hp == 0:
                            emit_v(2 * kcp)
                            emit_v(2 * kcp + 1)
                        ests = [
                            big.tile([P, 1024], ADT, tag="big",
                                     name=f"est{qg}_{hp}_{kcp}_{s}")
                            for s in range(2)
                        ]
                        # S^T chunks [128 keys, 512 queries] (K=64), sub0/
                        # sub1 interleaved: consecutive matmuls target
                        # different PE row groups and can run concurrently.
                        # One [128,512] psum chunk per (sub, half) so the
                        # exp drains release slots at mm granularity.
                        for half in range(2):
                            kc = kcp * 2 + half
                            stps = [
                                st_ps.tile([P, 512], F32, tag="st", name="stp")
                                for _ in range(2)
                            ]
                            for sub in range(2):
                                nc.tensor.matmul(
                                    stps[sub],
                                    lhsT=kT[sub * dh : (sub + 1) * dh, hp, kc * P : (kc + 1) * P],
                                    rhs=qT[sub * dh : (sub + 1) * dh, hp, qs],
                                    start=True,
                                    stop=True,
                                    tile_position=(sub * dh, 0),
                                )
                            for sub in range(2):
                                dst = ests[sub][:, half * 512 : (half + 1) * 512]
                                if dve_unit[kcp * 2 + sub]:
                                    # Schraudolph fast-exp on DVE: one
                                    # mult-add into int16 bits, bitcast bf16.
                                    nc.vector.tensor_scalar(
                                        out=dst.bitcast(I16),
                                        in0=stps[sub],
                                        scalar1=EXP_A,
                                        scalar2=EXP_B,
                                        op0=mybir.AluOpType.mult,
                                        op1=mybir.AluOpType.add,
                                    )
                                else:
                                    nc.scalar.activation(
                                        out=dst,
                                        in_=stps[sub],
                                        func=mybir.ActivationFunctionType.Exp,
                                    )
                        for half in range(2):
                            ov_q.append((ctx, kcp * 2 + half, ests, half))
                        while len(ov_q) > ov_delay:
                            do_ov(*ov_q.pop(0))
                        if pending_norm and (hp == 0 and kcp >= 5 or hp == 1):
                            emit_norm(*pending_norm.pop(0))
                        if hp == 1 and kcp % 2 == 1 and pending_outproj:
                            pqg, paot, pj = pending_outproj[0]
                            emit_outproj_j(pqg, paot, pj)
                            if pj == 3:
                                pending_outproj.pop(0)
                            else:
                                pending_outproj[0] = (pqg, paot, pj + 1)
            for item in ov_q:
                do_ov(*item)
            for item in pending_norm:
                emit_norm(*item)
            for pqg, paot, pj in pending_outproj:
                for j in range(pj, 4):
                    emit_outproj_j(pqg, paot, j)
    nc.finalize()
    return nc


_NC_CACHE = {}


def _get_nc(mode="v3"):
    if mode not in _NC_CACHE:
        _NC_CACHE[mode] = build_attention_kernel_v3()
    return _NC_CACHE[mode]


def shard_inputs(x, gamma, w_qkv, w_out, mode="v3"):
    """FULL inputs -> list of 8 per-core input maps.

    Host-side prep (fp64): gamma+1 and the 1/sqrt(dh) attention scale are
    folded into w_qkv; the per-token rms scale rs = sqrt(d)/||x_t|| is
    precomputed and shipped as a tiny [n] f32 tensor.
    """
    import ml_dtypes

    pdt = ml_dtypes.bfloat16
    d = x.shape[-1]
    dq = w_out.shape[0] // 4
    scale = DH ** -0.5
    gp1 = gamma.astype(np.float64) + 1.0
    w = w_qkv.astype(np.float64) * gp1[:, None]
    w[:, :d] *= scale  # q columns also absorb the softmax scale
    xs = x.astype(np.float64)
    rs = (d ** 0.5) / np.maximum(np.linalg.norm(xs, axis=-1), 1e-12)  # [b, n]
    xn = xs * rs[:, :, None]  # rms-normalized x (gamma fold lives in w)
    in_maps = []
    for c in range(NCORES):
        bi, g = c // 4, c % 4
        cs = slice(g * dq, (g + 1) * dq)
        wqkv_s = np.concatenate(
            [w[:, cs], w[:, d:][:, cs], w[:, 2 * d:][:, cs]], axis=1
        )
        xt = xn[bi].T.astype(pdt)  # [d, n]
        nhv = 4
        xt_tiled = np.ascontiguousarray(
            xt.reshape(d // P, P, nhv, x.shape[1] // nhv).transpose(2, 1, 0, 3)
        )
        wq = wqkv_s.astype(pdt)  # [d, 3*dq]
        wq_tiled = np.ascontiguousarray(
            wq.reshape(d // P, P, 3 * dq).transpose(1, 0, 2)
        )
        wo = w_out[cs, :].astype(pdt)  # [dq, d]
        wo_tiled = np.ascontiguousarray(
            wo.reshape(dq // P, P, d).transpose(1, 0, 2)
        )
        in_maps.append(
            {
                "xT": xt_tiled,
                "wqkv": wq_tiled,
                "wout": wo_tiled,
            }
        )
    return in_maps


def unshard_outputs(results):
    """8 partial [N, D] outputs -> full [B, N, D] (sum head groups per batch)."""
    outs = [np.asarray(r["out"], dtype=np.float32) for r in results]
    return np.stack(
        [
            outs[0] + outs[1] + outs[2] + outs[3],
            outs[4] + outs[5] + outs[6] + outs[7],
        ]
    ).astype(np.float32)


def run(x, gamma, w_qkv, w_out, mode="v3", **spmd_kwargs):
    nc = _get_nc(mode)
    in_maps = shard_inputs(x, gamma, w_qkv, w_out, mode)
    res = run_bass_kernel_spmd(nc, in_maps, list(range(NCORES)), **spmd_kwargs)
    return unshard_outputs(res.results), res


def kernel(x, gamma, w_qkv, w_out):
    out, _ = run(
        np.asarray(x), np.asarray(gamma), np.asarray(w_qkv), np.asarray(w_out)
    )
    return out


# revision 63
# speedup vs baseline: 1.1099x; 1.1099x over previous
"""Trainium2 Bass kernel for dense multi-head self-attention.

Reference computation (fp32):
    xn  = rms_norm(x) * (gamma + 1)          # F.normalize(x) * sqrt(D) * (gamma+1)
    qkv = xn @ w_qkv ; split into q, k, v    # heads H=16, dim_head 64
    out = softmax(q k^T / sqrt(64)) v
    y   = out @ w_out
Sharding (8 cores): data-parallel over batch (2), tensor-parallel over heads
(16 -> 4 groups of 4). Core c handles batch c//4, head group c%4. w_qkv is
column-sliced, w_out row-sliced per head group; each core emits a partial
[2048, 1024] output which the host sums per batch. No cross-device
communication inside the kernel.

v3 design notes (per-core, all bf16 matmuls, fp32 PSUM accumulation):
  - gamma+1 and the 1/sqrt(64) q-scale are folded into w_qkv on the HOST
    (fp64), and the rms scale rs[t] = sqrt(d)/||x_t|| (0.02% of the FLOPs)
    is computed on the host and shipped as a [n] f32 input. This removes
    every ACT function except Exp -> exactly one ACT_TABLE_LOAD, which
    matters because Ln/Sqrt/Reciprocal live in different table sets and
    each switch costs ~2.7us of serialized ACT time.
  - softmax exp of S^T chunks is SPLIT between ACT (true Exp) and DVE
    (Schraudolph fast-exp): bits = round(s*128/ln2 + (16256-5.51)) as
    int16, bitcast to bf16 => exp(s)*(1 +- 3%). One tensor_scalar op per
    [128,1024] chunk, psum->sbuf. The split fraction balances ACT vs DVE
    occupancy so the PE attention matmuls stay the critical path.
  - attention-out psum tiles are drained to SBUF by ACT copies (no table,
    psum-adjacent engine) as soon as each head-pair finishes, so the psum
    slots never wait on the softmax normalization chain (the v2 stall that
    caused ~3.5us PE gaps per head-pair and HAM clock-gate oscillation).
  - softmax denominators (ones-column of the V matmul) are ACT-copied onto
    4 separate partitions of a [4,512] pack per query group; ONE DVE
    reciprocal per qg costs 4.3us (DVE reciprocal is 8 cyc per FREE
    element, partition-parallel -- row-by-row it was 66us in the baseline).
  - normalization multiplies (attn_out * 1/denom broadcast) run on GPSIMD,
    which is otherwise idle; DVE is loaded with its exp share.
  - scores S^T per head-pair use tile_position row packing (dh=64), with
    the two heads' matmuls interleaved so consecutive PE instructions hit
    different row groups and run concurrently.
"""

import numpy as np

import concourse.bass as bass  # noqa: F401
import concourse.mybir as mybir
import concourse.tile as tile
from concourse import bacc
from concourse.bass_utils import run_bass_kernel_spmd

# Problem constants (hardcoded per contract; kernel.py must be self-contained).
B = 2          # batch
N = 2048       # sequence length
D = 1024       # model dim
H = 16         # total heads
DH = 64        # dim per head
HL = 4         # heads per core
DQ = HL * DH   # 256 = per-core q/k/v width
NCORES = 8

P = 128        # partitions

F32 = mybir.dt.float32
BF16 = mybir.dt.bfloat16
I16 = mybir.dt.int16

# Schraudolph fast-exp constants in bf16-exponent space:
#   bits = s * (2^7 / ln 2) + (127*2^7 - c);  bitcast<bf16>(bits) ~ exp(s)
# c = 2^7 * 0.043 balances the (1+f)/2^f linear-interp error to +-3%.
EXP_A = 128.0 / np.log(2.0)
EXP_B = 16256.0 - 5.513


def build_attention_kernel_v3(n=N, d=D, hl=HL, dh=DH, dve_sixteenths=7,
                              ov_delay=8):
    """Build the single-core SPMD Bass program (v3, all-bf16).

    dve_sixteenths: of the 16 (kcp, sub) exp units per (qg, hp), how many
    route to the DVE fast-exp instead of ACT Exp.
    """
    PDT = BF16
    ADT = BF16
    dq = hl * dh
    ndc = d // P        # dim chunks of 128
    nt4 = n // 512      # token tiles of 512
    nt16 = n // P       # token tiles of 128
    kc_n = n // P       # key chunks of 128
    qg_n = n // 512     # query groups of 512
    hp_n = hl // 2      # head pairs

    # Routing of the 16 exp units (kcp 0..7 x sub 0..1) to DVE: strict
    # sub-alternation (sub0 -> ACT, sub1 -> DVE) so every kcp's exp wall is
    # max(570, 659) < the PE time per kcp -- except kcp5, whose DVE unit
    # moves to ACT; the per-qg DVE reciprocal is scheduled exactly there so
    # it never delays a fast-exp the scores are waiting on.
    dve_unit = [(u % 2 == 1) and u != 11 for u in range(16)]

    nc = bacc.Bacc()
    # xT arrives already rms-normalized (host folds rs[t] = sqrt(d)/||x_t||
    # into the columns), so the q/k/v psum drains are plain copies. All
    # inputs are HOST-PRE-TILED to the on-chip layout so every DMA reads
    # long contiguous runs (the naive row-strided patterns cost ~10ns per
    # descriptor -- the wqkv load alone was 1024 descriptors = 10us on the
    # first-matmul critical path).
    n_halves_ = 4 if n >= 2048 else (2 if n >= 1024 else 1)
    xT_d = nc.declare_dram_parameter(
        "xT", [n_halves_, P, d // P, n // n_halves_], PDT, isOutput=False
    )
    wqkv_d = nc.declare_dram_parameter(
        "wqkv", [P, d // P, 3 * dq], PDT, isOutput=False
    )
    wout_d = nc.declare_dram_parameter(
        "wout", [P, dq // P, d], PDT, isOutput=False
    )
    # Partial outputs in bf16 (host sums the head groups in f32): halves
    # the store traffic; quantization is ~0.2% of the summed output.
    out_d = nc.declare_dram_parameter("out", [n, d], BF16, isOutput=True)

    kc2_n = dq // P     # contraction chunks for the output projection
    on_n = d // 512     # output-column tiles
    n_halves = 4 if n >= 2048 else (2 if n >= 1024 else 1)
    nh = n // n_halves

    with tile.TileContext(nc) as tc:
        with (
            # 2 KiB/partition slots for the expS^T tiles.
            tc.tile_pool(name="big", bufs=16) as big,
            tc.tile_pool(name="consts", bufs=1) as consts,
            tc.tile_pool(name="weights", bufs=1) as weights,
            tc.tile_pool(name="qkt", bufs=1) as qkt,
            tc.tile_pool(name="vpool", bufs=1) as vpool,
            tc.tile_pool(name="otc", bufs=2) as otc_pool,
            tc.tile_pool(name="recip", bufs=2) as recip,
            tc.tile_pool(name="aot", bufs=2) as aot_pool,
            tc.tile_pool(name="outsb", bufs=3) as outsb,
            tc.tile_pool(name="st_ps", bufs=4, space="PSUM") as st_ps,
            tc.tile_pool(name="ot_ps", bufs=2, space="PSUM") as ot_ps,
            tc.tile_pool(name="proj_ps", bufs=2, space="PSUM") as proj_ps,
        ):
            # wqkv on the ACT hwdge queue so the x tiles stream on the SP
            # queue concurrently from t=0.
            # wqkv: q|k columns first (they gate the first matmul), the v
            # columns behind the first x half on the other queue.
            wqkv_sb = weights.tile([P, ndc, 3 * dq], PDT, tag="wqkv")
            nc.scalar.dma_start(
                out=wqkv_sb[:, :, 0 : 2 * dq], in_=wqkv_d[:, :, 0 : 2 * dq]
            )
            xbig = consts.tile([P, ndc, n], PDT, tag="xbig")
            for h2 in range(n_halves):
                for dcg in range(2):  # split each half over the two queues
                    eng = nc.sync if (h2 + dcg) % 2 == 0 else nc.scalar
                    eng.dma_start(
                        out=xbig[:, dcg * ndc // 2 : (dcg + 1) * ndc // 2,
                                 h2 * nh : (h2 + 1) * nh],
                        in_=xT_d[h2, :, dcg * ndc // 2 : (dcg + 1) * ndc // 2],
                    )
                if h2 == 0:
                    nc.scalar.dma_start(
                        out=wqkv_sb[:, :, 2 * dq : 3 * dq],
                        in_=wqkv_d[:, :, 2 * dq : 3 * dq],
                    )
            # Late load: only needed by the output projection.
            wout_sb = weights.tile([P, kc2_n, d], PDT, tag="wout")
            nc.scalar.dma_start(out=wout_sb, in_=wout_d[:])

            def xt_slice(dc, lo, size):
                return xbig[:, dc, lo : lo + size]

            ones_bf = consts.tile([P, nt16 * hl], PDT, tag="ones_bf")
            nc.vector.memset(ones_bf, 1.0)

            # q^T / k^T projections: [128 rows = head-pair x 64 dims, tokens].
            # rms normalization (rs per token) applied at the psum drain.
            qT = qkt.tile([P, hp_n, n], ADT, tag="qT")
            kT = qkt.tile([P, hp_n, n], ADT, tag="kT")
            for h2 in range(n_halves):
                for hp in range(hp_n):
                    for part in range(2):  # 0 = q, 1 = k
                        for nt in range(h2 * nt4 // n_halves, (h2 + 1) * nt4 // n_halves):
                            ps = proj_ps.tile([P, 512], F32, tag="proj", name="psqk")
                            off = part * dq + hp * P
                            for dc in range(ndc):
                                nc.tensor.matmul(
                                    ps,
                                    lhsT=wqkv_sb[:, dc, off : off + P],
                                    rhs=xt_slice(dc, nt * 512, 512),
                                    start=(dc == 0),
                                    stop=(dc == ndc - 1),
                                )
                            dst = qT if part == 0 else kT
                            nc.vector.tensor_copy(
                                dst[:, hp, nt * 512 : (nt + 1) * 512], ps
                            )

            # v projection in natural orientation [token, head*dh], with a
            # ones column appended per head (softmax denominator trick).
            # The 16 v tiles are emitted interleaved into the first query
            # group's score stream (the OV pipeline consumes v_sb only
            # ov_delay chunks behind), so the exp engines start earlier.
            v_sb = vpool.tile([P, nt16, hl, dh + 1], ADT, tag="v")
            nc.vector.tensor_copy(
                v_sb[:, :, :, dh : dh + 1].rearrange("p a b o -> p (a b o)"),
                ones_bf,
            )

            def emit_v(ntt):
                ps = proj_ps.tile([P, dq], F32, tag="proj", name="psv")
                for dc in range(ndc):
                    nc.tensor.matmul(
                        ps,
                        lhsT=xt_slice(dc, ntt * P, P),
                        rhs=wqkv_sb[:, dc, 2 * dq : 3 * dq],
                        start=(dc == 0),
                        stop=(dc == ndc - 1),
                    )
                nc.vector.tensor_copy(
                    v_sb[:, ntt, :, 0:dh],
                    ps.rearrange("p (h dd) -> p h dd", h=hl),
                )

            # Attention + output projection, one query group (512) at a
            # time, software-pipelined across engines:
            #   PE:     scores (row-packed head pair) -> OV (lagged ov_delay)
            #   ACT:    Exp of (1 - alpha) of the S^T chunks; psum->sbuf
            #           drains of finished attention-out tiles
            #   DVE:    fast-exp of alpha of the chunks; denominator recip
            #   GPSIMD: 1/denom broadcast + normalize multiply into aot
            out_ap = out_d[:]
            pending_otcopy = []
            pending_norm = []
            pending_outproj = []

            def emit_otcopy(qg, hp, ots, otc, dpk):
                # Drain attention-out psum [65,512] per sub into the otc
                # staging tile (one ACT copy; the denominator row 64 rides
                # along and is picked up by the norm DMA from otc[64]).
                for sub in range(2):
                    u = hp * 2 + sub
                    nc.scalar.copy(otc[:, u, :], ots[sub][0 : dh + 1, :])

            def emit_norm(qg, otc, dpk, aot, hps=(0, 1)):
                # Denominator rows sit side by side on partition 64 of otc;
                # SBUF->SBUF DMA spreads them over separate partitions so
                # ONE DVE reciprocal covers them lane-parallel (reciprocal
                # costs 8 cyc per FREE element), then a DMA brings the
                # results back to partition 0 for the gpsimd broadcasts.
                us = [hp * 2 + s for hp in hps for s in range(2)]
                nu = len(us)
                dpn = recip.tile([nu, 512], F32, tag="dp4", name=f"dp{qg}_{us[0]}")
                nc.scalar.dma_start(
                    out=dpn,
                    in_=otc[dh : dh + 1, us[0] : us[0] + nu, :].rearrange(
                        "o u t -> o (u t)"
                    ),
                )
                rrn = recip.tile([nu, 512], F32, tag="rr4", name=f"rr{qg}_{us[0]}")
                nc.vector.reciprocal(rrn, dpn)
                rrow = recip.tile([1, nu, 512], F32, tag="rrow", name=f"rw{qg}_{us[0]}")
                nc.scalar.dma_start(
                    out=rrow[0:1].rearrange("o u t -> o (u t)"),
                    in_=rrn,
                )
                for i, u in enumerate(us):
                    hp, sub = u // 2, u % 2
                    rb = recip.tile([dh, 512], F32, tag="rbcast", name="rb")
                    nc.gpsimd.partition_broadcast(rb, rrow[0:1, i, :], channels=dh)
                    nc.vector.tensor_mul(
                        out=aot[sub * dh : (sub + 1) * dh, hp, :],
                        in0=otc[0:dh, u, :],
                        in1=rb,
                    )

            def emit_outproj_j(qg, aot, j):
                # One 128-token row block: kc2-major matmul order reuses
                # each stationary for both output-column tiles, halving the
                # weight loads; drains alternate DVE/ACT to avoid bursts.
                ntt = qg * 4 + j
                pss = [
                    proj_ps.tile([P, 512], F32, tag="proj", name="pso")
                    for _ in range(on_n)
                ]
                for kc2 in range(kc2_n):
                    for on in range(on_n):
                        nc.tensor.matmul(
                            pss[on],
                            lhsT=aot[:, kc2, j * P : (j + 1) * P],
                            rhs=wout_sb[:, kc2, on * 512 : (on + 1) * 512],
                            start=(kc2 == 0),
                            stop=(kc2 == kc2_n - 1),
                        )
                for on in range(on_n):
                    ob = outsb.tile([P, 512], BF16, tag="outsb", name="ob")
                    if on % 2 == 0:
                        nc.vector.tensor_copy(ob, pss[on])
                    else:
                        nc.scalar.copy(ob, pss[on])
                    eng = nc.sync if (j + on) % 2 == 0 else nc.scalar
                    eng.dma_start(
                        out=out_ap[ntt * P : (ntt + 1) * P, on * 512 : (on + 1) * 512],
                        in_=ob,
                    )

            def emit_outproj(qg, aot):
                for j in range(4):
                    emit_outproj_j(qg, aot, j)

            # The OV queue carries across head-pair and query-group
            # boundaries: while the tail OVs of one block wait on their exp
            # results, the next block's score matmuls keep the PE busy.
            ov_q = []

            def do_ov(ctx, kc, ests, half):
                qg, hp, ots, otc, dpk, aot = ctx
                for sub in range(2):
                    nc.tensor.matmul(
                        ots[sub],
                        lhsT=v_sb[:, kc, hp * 2 + sub, :],
                        rhs=ests[sub][:, half * 512 : (half + 1) * 512],
                        start=(kc == 0),
                        stop=(kc == kc_n - 1),
                    )
                if kc == kc_n - 1:
                    # Head pair finished: free the psum slots immediately
                    # (ACT copies), queue normalization work per qg. The
                    # LAST qg normalizes per head-pair and projects inline
                    # so the end-of-kernel flush chain is short.
                    emit_otcopy(qg, hp, ots, otc, dpk)
                    if qg == qg_n - 1:
                        emit_norm(qg, otc, dpk, aot, hps=(hp,))
                        if hp == hp_n - 1:
                            emit_outproj(qg, aot)
                    elif hp == hp_n - 1:
                        pending_norm.append((qg, otc, dpk, aot))
                        pending_outproj.append((qg, aot, 0))

            for qg in range(qg_n):
                qs = slice(qg * 512, (qg + 1) * 512)
                aot = aot_pool.tile([P, kc2_n, 512], PDT, tag="aot", name=f"aot{qg}")
                otc = otc_pool.tile([dh + 1, 4, 512], F32, tag="otc", name=f"otc{qg}")
                dpk = None
                for hp in range(hp_n):
                    ots = [
                        ot_ps.tile([dh + 1, 512], F32, tag="ot", name=f"ot{qg}_{hp}_{s}")
                        for s in range(2)
                    ]
                    ctx = (qg, hp, ots, otc, dpk, aot)
                    for kcp in range(kc_n // 2):
                        if qg == 0 and hp == 0:
                            emit_v(2 * kcp)
                            emit_v(2 * kcp + 1)
                        ests = [
                            big.tile([P, 1024], ADT, tag="big",
                                     name=f"est{qg}_{hp}_{kcp}_{s}")
                            for s in range(2)
                        ]
                        # S^T chunks [128 keys, 512 queries] (K=64), sub0/
                        # sub1 interleaved: consecutive matmuls target
                        # different PE row groups and can run concurrently.
                        # One [128,512] psum chunk per (sub, half) so the
                        # exp drains release slots at mm granularity.
                        for half in range(2):
                            kc = kcp * 2 + half
                            stps = [
                                st_ps.tile([P, 512], F32, tag="st", name="stp")
                                for _ in range(2)
                            ]
                            for sub in range(2):
                                nc.tensor.matmul(
                                    stps[sub],
                                    lhsT=kT[sub * dh : (sub + 1) * dh, hp, kc * P : (kc + 1) * P],
                                    rhs=qT[sub * dh : (sub + 1) * dh, hp, qs],
                                    start=True,
                                    stop=True,
                                    tile_position=(sub * dh, 0),
                                )
                            for sub in range(2):
                                dst = ests[sub][:, half * 512 : (half + 1) * 512]
                                if dve_unit[kcp * 2 + sub]:
                                    # Schraudolph fast-exp on DVE: one
                                    # mult-add into int16 bits, bitcast bf16.
                                    nc.vector.tensor_scalar(
                                        out=dst.bitcast(I16),
                                        in0=stps[sub],
                                        scalar1=EXP_A,
                                        scalar2=EXP_B,
                                        op0=mybir.AluOpType.mult,
                                        op1=mybir.AluOpType.add,
                                    )
                                else:
                                    nc.scalar.activation(
                                        out=dst,
                                        in_=stps[sub],
                                        func=mybir.ActivationFunctionType.Exp,
                                    )
                        for half in range(2):
                            ov_q.append((ctx, kcp * 2 + half, ests, half))
                        while len(ov_q) > ov_delay:
                            do_ov(*ov_q.pop(0))
                        if pending_norm and (hp == 0 and kcp >= 5 or hp == 1):
                            emit_norm(*pending_norm.pop(0))
                        if hp == 1 and kcp % 2 == 1 and pending_outproj:
                            pqg, paot, pj = pending_outproj[0]
                            emit_outproj_j(pqg, paot, pj)
                            if pj == 3:
                                pending_outproj.pop(0)
                            else:
                                pending_outproj[0] = (pqg, paot, pj + 1)
            for item in ov_q:
                do_ov(*item)
            for item in pending_norm:
                emit_norm(*item)
            for pqg, paot, pj in pending_outproj:
                for j in range(pj, 4):
                    emit_outproj_j(pqg, paot, j)
    nc.finalize()
    return nc


_NC_CACHE = {}


def _get_nc(mode="v3"):
    if mode not in _NC_CACHE:
        _NC_CACHE[mode] = build_attention_kernel_v3()
    return _NC_CACHE[mode]


def shard_inputs(x, gamma, w_qkv, w_out, mode="v3"):
    """FULL inputs -> list of 8 per-core input maps.

    Host-side prep (fp64): gamma+1 and the 1/sqrt(dh) attention scale are
    folded into w_qkv; the per-token rms scale rs = sqrt(d)/||x_t|| is
    precomputed and shipped as a tiny [n] f32 tensor.
    """
    import ml_dtypes

    pdt = ml_dtypes.bfloat16
    d = x.shape[-1]
    dq = w_out.shape[0] // 4
    scale = DH ** -0.5
    gp1 = gamma.astype(np.float64) + 1.0
    w = w_qkv.astype(np.float64) * gp1[:, None]
    w[:, :d] *= scale  # q columns also absorb the softmax scale
    xs = x.astype(np.float64)
    rs = (d ** 0.5) / np.maximum(np.linalg.norm(xs, axis=-1), 1e-12)  # [b, n]
    xn = xs * rs[:, :, None]  # rms-normalized x (gamma fold lives in w)
    in_maps = []
    for c in range(NCORES):
        bi, g = c // 4, c % 4
        cs = slice(g * dq, (g + 1) * dq)
        wqkv_s = np.concatenate(
            [w[:, cs], w[:, d:][:, cs], w[:, 2 * d:][:, cs]], axis=1
        )
        xt = xn[bi].T.astype(pdt)  # [d, n]
        nhv = 4
        xt_tiled = np.ascontiguousarray(
            xt.reshape(d // P, P, nhv, x.shape[1] // nhv).transpose(2, 1, 0, 3)
        )
        wq = wqkv_s.astype(pdt)  # [d, 3*dq]
        wq_tiled = np.ascontiguousarray(
            wq.reshape(d // P, P, 3 * dq).transpose(1, 0, 2)
        )
        wo = w_out[cs, :].astype(pdt)  # [dq, d]
        wo_tiled = np.ascontiguousarray(
            wo.reshape(dq // P, P, d).transpose(1, 0, 2)
        )
        in_maps.append(
            {
                "xT": xt_tiled,
                "wqkv": wq_tiled,
                "wout": wo_tiled,
            }
        )
    return in_maps


def unshard_outputs(results):
    """8 partial [N, D] outputs -> full [B, N, D] (sum head groups per batch)."""
    outs = [np.asarray(r["out"], dtype=np.float32) for r in results]
    return np.stack(
        [
            outs[0] + outs[1] + outs[2] + outs[3],
            outs[4] + outs[5] + outs[6] + outs[7],
        ]
    ).astype(np.float32)


def run(x, gamma, w_qkv, w_out, mode="v3", **spmd_kwargs):
    nc = _get_nc(mode)
    in_maps = shard_inputs(x, gamma, w_qkv, w_out, mode)
    res = run_bass_kernel_spmd(nc, in_maps, list(range(NCORES)), **spmd_kwargs)
    return unshard_outputs(res.results), res


def kernel(x, gamma, w_qkv, w_out):
    out, _ = run(
        np.asarray(x), np.asarray(gamma), np.asarray(w_qkv), np.asarray(w_out)
    )
    return out


# revision 64
# speedup vs baseline: 1.1201x; 1.0091x over previous
"""Trainium2 Bass kernel for dense multi-head self-attention.

Reference computation (fp32):
    xn  = rms_norm(x) * (gamma + 1)          # F.normalize(x) * sqrt(D) * (gamma+1)
    qkv = xn @ w_qkv ; split into q, k, v    # heads H=16, dim_head 64
    out = softmax(q k^T / sqrt(64)) v
    y   = out @ w_out
Sharding (8 cores): data-parallel over batch (2), tensor-parallel over heads
(16 -> 4 groups of 4). Core c handles batch c//4, head group c%4. w_qkv is
column-sliced, w_out row-sliced per head group; each core emits a partial
[2048, 1024] output which the host sums per batch. No cross-device
communication inside the kernel.

v3 design notes (per-core, all bf16 matmuls, fp32 PSUM accumulation):
  - gamma+1 and the 1/sqrt(64) q-scale are folded into w_qkv on the HOST
    (fp64), and the rms scale rs[t] = sqrt(d)/||x_t|| (0.02% of the FLOPs)
    is computed on the host and shipped as a [n] f32 input. This removes
    every ACT function except Exp -> exactly one ACT_TABLE_LOAD, which
    matters because Ln/Sqrt/Reciprocal live in different table sets and
    each switch costs ~2.7us of serialized ACT time.
  - softmax exp of S^T chunks is SPLIT between ACT (true Exp) and DVE
    (Schraudolph fast-exp): bits = round(s*128/ln2 + (16256-5.51)) as
    int16, bitcast to bf16 => exp(s)*(1 +- 3%). One tensor_scalar op per
    [128,1024] chunk, psum->sbuf. The split fraction balances ACT vs DVE
    occupancy so the PE attention matmuls stay the critical path.
  - attention-out psum tiles are drained to SBUF by ACT copies (no table,
    psum-adjacent engine) as soon as each head-pair finishes, so the psum
    slots never wait on the softmax normalization chain (the v2 stall that
    caused ~3.5us PE gaps per head-pair and HAM clock-gate oscillation).
  - softmax denominators (ones-column of the V matmul) are ACT-copied onto
    4 separate partitions of a [4,512] pack per query group; ONE DVE
    reciprocal per qg costs 4.3us (DVE reciprocal is 8 cyc per FREE
    element, partition-parallel -- row-by-row it was 66us in the baseline).
  - normalization multiplies (attn_out * 1/denom broadcast) run on GPSIMD,
    which is otherwise idle; DVE is loaded with its exp share.
  - scores S^T per head-pair use tile_position row packing (dh=64), with
    the two heads' matmuls interleaved so consecutive PE instructions hit
    different row groups and run concurrently.
"""

import numpy as np

import concourse.bass as bass  # noqa: F401
import concourse.mybir as mybir
import concourse.tile as tile
from concourse import bacc
from concourse.bass_utils import run_bass_kernel_spmd

# Problem constants (hardcoded per contract; kernel.py must be self-contained).
B = 2          # batch
N = 2048       # sequence length
D = 1024       # model dim
H = 16         # total heads
DH = 64        # dim per head
HL = 4         # heads per core
DQ = HL * DH   # 256 = per-core q/k/v width
NCORES = 8

P = 128        # partitions

F32 = mybir.dt.float32
BF16 = mybir.dt.bfloat16
I16 = mybir.dt.int16

# Schraudolph fast-exp constants in bf16-exponent space:
#   bits = s * (2^7 / ln 2) + (127*2^7 - c);  bitcast<bf16>(bits) ~ exp(s)
# c = 2^7 * 0.043 balances the (1+f)/2^f linear-interp error to +-3%.
EXP_A = 128.0 / np.log(2.0)
EXP_B = 16256.0 - 5.513


def build_attention_kernel_v3(n=N, d=D, hl=HL, dh=DH, dve_sixteenths=7,
                              ov_delay=8):
    """Build the single-core SPMD Bass program (v3, all-bf16).

    dve_sixteenths: of the 16 (kcp, sub) exp units per (qg, hp), how many
    route to the DVE fast-exp instead of ACT Exp.
    """
    PDT = BF16
    ADT = BF16
    dq = hl * dh
    ndc = d // P        # dim chunks of 128
    nt4 = n // 512      # token tiles of 512
    nt16 = n // P       # token tiles of 128
    kc_n = n // P       # key chunks of 128
    qg_n = n // 512     # query groups of 512
    hp_n = hl // 2      # head pairs

    # Routing of the 16 exp units (kcp 0..7 x sub 0..1) to DVE: strict
    # sub-alternation (sub0 -> ACT, sub1 -> DVE) so every kcp's exp wall is
    # max(570, 659) < the PE time per kcp -- except kcp5, whose DVE unit
    # moves to ACT; the per-qg DVE reciprocal is scheduled exactly there so
    # it never delays a fast-exp the scores are waiting on.
    dve_unit = [(u % 2 == 1) and u != 11 for u in range(16)]

    nc = bacc.Bacc()
    # xT arrives already rms-normalized (host folds rs[t] = sqrt(d)/||x_t||
    # into the columns), so the q/k/v psum drains are plain copies. All
    # inputs are HOST-PRE-TILED to the on-chip layout so every DMA reads
    # long contiguous runs (the naive row-strided patterns cost ~10ns per
    # descriptor -- the wqkv load alone was 1024 descriptors = 10us on the
    # first-matmul critical path).
    n_halves_ = 4 if n >= 2048 else (2 if n >= 1024 else 1)
    xT_d = nc.declare_dram_parameter(
        "xT", [n_halves_, P, d // P, n // n_halves_], PDT, isOutput=False
    )
    wqkv_d = nc.declare_dram_parameter(
        "wqkv", [P, d // P, 3 * dq], PDT, isOutput=False
    )
    wout_d = nc.declare_dram_parameter(
        "wout", [P, dq // P, d], PDT, isOutput=False
    )
    # Partial outputs in bf16 (host sums the head groups in f32): halves
    # the store traffic; quantization is ~0.2% of the summed output.
    out_d = nc.declare_dram_parameter("out", [n, d], BF16, isOutput=True)

    kc2_n = dq // P     # contraction chunks for the output projection
    on_n = d // 512     # output-column tiles
    n_halves = 4 if n >= 2048 else (2 if n >= 1024 else 1)
    nh = n // n_halves

    with tile.TileContext(nc) as tc:
        with (
            # 2 KiB/partition slots for the expS^T tiles. Deeper than the
            # OV lag so the est ring never back-pressures the exp drains.
            tc.tile_pool(name="big", bufs=24) as big,
            tc.tile_pool(name="consts", bufs=1) as consts,
            tc.tile_pool(name="weights", bufs=1) as weights,
            tc.tile_pool(name="qkt", bufs=1) as qkt,
            tc.tile_pool(name="vpool", bufs=1) as vpool,
            tc.tile_pool(name="otc", bufs=2) as otc_pool,
            tc.tile_pool(name="recip", bufs=2) as recip,
            tc.tile_pool(name="aot", bufs=2) as aot_pool,
            tc.tile_pool(name="outsb", bufs=3) as outsb,
            tc.tile_pool(name="st_ps", bufs=4, space="PSUM") as st_ps,
            tc.tile_pool(name="ot_ps", bufs=2, space="PSUM") as ot_ps,
            tc.tile_pool(name="proj_ps", bufs=2, space="PSUM") as proj_ps,
        ):
            # wqkv on the ACT hwdge queue so the x tiles stream on the SP
            # queue concurrently from t=0.
            # wqkv: q|k columns first (they gate the first matmul), the v
            # columns behind the first x half on the other queue.
            wqkv_sb = weights.tile([P, ndc, 3 * dq], PDT, tag="wqkv")
            nc.scalar.dma_start(
                out=wqkv_sb[:, :, 0 : 2 * dq], in_=wqkv_d[:, :, 0 : 2 * dq]
            )
            xbig = consts.tile([P, ndc, n], PDT, tag="xbig")
            for h2 in range(n_halves):
                for dcg in range(2):  # split each half over the two queues
                    eng = nc.sync if (h2 + dcg) % 2 == 0 else nc.scalar
                    eng.dma_start(
                        out=xbig[:, dcg * ndc // 2 : (dcg + 1) * ndc // 2,
                                 h2 * nh : (h2 + 1) * nh],
                        in_=xT_d[h2, :, dcg * ndc // 2 : (dcg + 1) * ndc // 2],
                    )
                if h2 == 0:
                    nc.scalar.dma_start(
                        out=wqkv_sb[:, :, 2 * dq : 3 * dq],
                        in_=wqkv_d[:, :, 2 * dq : 3 * dq],
                    )
            # Late load: only needed by the output projection.
            wout_sb = weights.tile([P, kc2_n, d], PDT, tag="wout")
            nc.scalar.dma_start(out=wout_sb, in_=wout_d[:])

            def xt_slice(dc, lo, size):
                return xbig[:, dc, lo : lo + size]

            ones_bf = consts.tile([P, nt16 * hl], PDT, tag="ones_bf")
            nc.vector.memset(ones_bf, 1.0)

            # q^T / k^T projections: [128 rows = head-pair x 64 dims, tokens].
            # rms normalization (rs per token) applied at the psum drain.
            qT = qkt.tile([P, hp_n, n], ADT, tag="qT")
            kT = qkt.tile([P, hp_n, n], ADT, tag="kT")
            for h2 in range(n_halves):
                for hp in range(hp_n):
                    for part in range(2):  # 0 = q, 1 = k
                        for nt in range(h2 * nt4 // n_halves, (h2 + 1) * nt4 // n_halves):
                            ps = proj_ps.tile([P, 512], F32, tag="proj", name="psqk")
                            off = part * dq + hp * P
                            for dc in range(ndc):
                                nc.tensor.matmul(
                                    ps,
                                    lhsT=wqkv_sb[:, dc, off : off + P],
                                    rhs=xt_slice(dc, nt * 512, 512),
                                    start=(dc == 0),
                                    stop=(dc == ndc - 1),
                                )
                            dst = qT if part == 0 else kT
                            nc.vector.tensor_copy(
                                dst[:, hp, nt * 512 : (nt + 1) * 512], ps
                            )

            # v projection in natural orientation [token, head*dh], with a
            # ones column appended per head (softmax denominator trick).
            # The 16 v tiles are emitted interleaved into the first query
            # group's score stream (the OV pipeline consumes v_sb only
            # ov_delay chunks behind), so the exp engines start earlier.
            v_sb = vpool.tile([P, nt16, hl, dh + 1], ADT, tag="v")
            nc.vector.tensor_copy(
                v_sb[:, :, :, dh : dh + 1].rearrange("p a b o -> p (a b o)"),
                ones_bf,
            )

            def emit_v(ntt):
                ps = proj_ps.tile([P, dq], F32, tag="proj", name="psv")
                for dc in range(ndc):
                    nc.tensor.matmul(
                        ps,
                        lhsT=xt_slice(dc, ntt * P, P),
                        rhs=wqkv_sb[:, dc, 2 * dq : 3 * dq],
                        start=(dc == 0),
                        stop=(dc == ndc - 1),
                    )
                nc.vector.tensor_copy(
                    v_sb[:, ntt, :, 0:dh],
                    ps.rearrange("p (h dd) -> p h dd", h=hl),
                )

            # Attention + output projection, one query group (512) at a
            # time, software-pipelined across engines:
            #   PE:     scores (row-packed head pair) -> OV (lagged ov_delay)
            #   ACT:    Exp of (1 - alpha) of the S^T chunks; psum->sbuf
            #           drains of finished attention-out tiles
            #   DVE:    fast-exp of alpha of the chunks; denominator recip
            #   GPSIMD: 1/denom broadcast + normalize multiply into aot
            out_ap = out_d[:]
            pending_otcopy = []
            pending_norm = []
            pending_outproj = []

            def emit_otcopy(qg, hp, ots, otc, dpk):
                # Drain attention-out psum [65,512] per sub into the otc
                # staging tile (one ACT copy; the denominator row 64 rides
                # along and is picked up by the norm DMA from otc[64]).
                for sub in range(2):
                    u = hp * 2 + sub
                    nc.scalar.copy(otc[:, u, :], ots[sub][0 : dh + 1, :])

            def emit_norm(qg, otc, dpk, aot, hps=(0, 1)):
                # Denominator rows sit side by side on partition 64 of otc;
                # SBUF->SBUF DMA spreads them over separate partitions so
                # ONE DVE reciprocal covers them lane-parallel (reciprocal
                # costs 8 cyc per FREE element), then a DMA brings the
                # results back to partition 0 for the gpsimd broadcasts.
                us = [hp * 2 + s for hp in hps for s in range(2)]
                nu = len(us)
                dpn = recip.tile([nu, 512], F32, tag="dp4", name=f"dp{qg}_{us[0]}")
                nc.scalar.dma_start(
                    out=dpn,
                    in_=otc[dh : dh + 1, us[0] : us[0] + nu, :].rearrange(
                        "o u t -> o (u t)"
                    ),
                )
                rrn = recip.tile([nu, 512], F32, tag="rr4", name=f"rr{qg}_{us[0]}")
                nc.vector.reciprocal(rrn, dpn)
                rrow = recip.tile([1, nu, 512], F32, tag="rrow", name=f"rw{qg}_{us[0]}")
                nc.scalar.dma_start(
                    out=rrow[0:1].rearrange("o u t -> o (u t)"),
                    in_=rrn,
                )
                for i, u in enumerate(us):
                    hp, sub = u // 2, u % 2
                    rb = recip.tile([dh, 512], F32, tag="rbcast", name="rb")
                    nc.gpsimd.partition_broadcast(rb, rrow[0:1, i, :], channels=dh)
                    nc.vector.tensor_mul(
                        out=aot[sub * dh : (sub + 1) * dh, hp, :],
                        in0=otc[0:dh, u, :],
                        in1=rb,
                    )

            def emit_outproj_j(qg, aot, j):
                # One 128-token row block: kc2-major matmul order reuses
                # each stationary for both output-column tiles, halving the
                # weight loads; drains alternate DVE/ACT to avoid bursts.
                ntt = qg * 4 + j
                pss = [
                    proj_ps.tile([P, 512], F32, tag="proj", name="pso")
                    for _ in range(on_n)
                ]
                for kc2 in range(kc2_n):
                    for on in range(on_n):
                        nc.tensor.matmul(
                            pss[on],
                            lhsT=aot[:, kc2, j * P : (j + 1) * P],
                            rhs=wout_sb[:, kc2, on * 512 : (on + 1) * 512],
                            start=(kc2 == 0),
                            stop=(kc2 == kc2_n - 1),
                        )
                for on in range(on_n):
                    ob = outsb.tile([P, 512], BF16, tag="outsb", name="ob")
                    if on % 2 == 0:
                        nc.vector.tensor_copy(ob, pss[on])
                    else:
                        nc.scalar.copy(ob, pss[on])
                    eng = nc.sync if (j + on) % 2 == 0 else nc.scalar
                    eng.dma_start(
                        out=out_ap[ntt * P : (ntt + 1) * P, on * 512 : (on + 1) * 512],
                        in_=ob,
                    )

            def emit_outproj(qg, aot):
                for j in range(4):
                    emit_outproj_j(qg, aot, j)

            # The OV queue carries across head-pair and query-group
            # boundaries: while the tail OVs of one block wait on their exp
            # results, the next block's score matmuls keep the PE busy.
            ov_q = []

            def do_ov(ctx, kc, ests, half):
                qg, hp, ots, otc, dpk, aot = ctx
                for sub in range(2):
                    nc.tensor.matmul(
                        ots[sub],
                        lhsT=v_sb[:, kc, hp * 2 + sub, :],
                        rhs=ests[sub][:, half * 512 : (half + 1) * 512],
                        start=(kc == 0),
                        stop=(kc == kc_n - 1),
                    )
                if kc == kc_n - 1:
                    # Head pair finished: free the psum slots immediately
                    # (ACT copies), queue normalization work per qg. The
                    # LAST qg normalizes per head-pair and projects inline
                    # so the end-of-kernel flush chain is short.
                    emit_otcopy(qg, hp, ots, otc, dpk)
                    if qg == qg_n - 1:
                        emit_norm(qg, otc, dpk, aot, hps=(hp,))
                        if hp == hp_n - 1:
                            emit_outproj(qg, aot)
                    elif hp == hp_n - 1:
                        pending_norm.append((qg, otc, dpk, aot))
                        pending_outproj.append((qg, aot, 0))

            for qg in range(qg_n):
                qs = slice(qg * 512, (qg + 1) * 512)
                aot = aot_pool.tile([P, kc2_n, 512], PDT, tag="aot", name=f"aot{qg}")
                otc = otc_pool.tile([dh + 1, 4, 512], F32, tag="otc", name=f"otc{qg}")
                dpk = None
                for hp in range(hp_n):
                    ots = [
                        ot_ps.tile([dh + 1, 512], F32, tag="ot", name=f"ot{qg}_{hp}_{s}")
                        for s in range(2)
                    ]
                    ctx = (qg, hp, ots, otc, dpk, aot)
                    for kcp in range(kc_n // 2):
                        if qg == 0 and hp == 0:
                            emit_v(2 * kcp)
                            emit_v(2 * kcp + 1)
                        ests = [
                            big.tile([P, 1024], ADT, tag="big",
                                     name=f"est{qg}_{hp}_{kcp}_{s}")
                            for s in range(2)
                        ]
                        # S^T chunks [128 keys, 512 queries] (K=64), sub0/
                        # sub1 interleaved: consecutive matmuls target
                        # different PE row groups and can run concurrently.
                        # One [128,512] psum chunk per (sub, half) so the
                        # exp drains release slots at mm granularity.
                        for half in range(2):
                            kc = kcp * 2 + half
                            stps = [
                                st_ps.tile([P, 512], F32, tag="st", name="stp")
                                for _ in range(2)
                            ]
                            for sub in range(2):
                                nc.tensor.matmul(
                                    stps[sub],
                                    lhsT=kT[sub * dh : (sub + 1) * dh, hp, kc * P : (kc + 1) * P],
                                    rhs=qT[sub * dh : (sub + 1) * dh, hp, qs],
                                    start=True,
                                    stop=True,
                                    tile_position=(sub * dh, 0),
                                )
                            for sub in range(2):
                                dst = ests[sub][:, half * 512 : (half + 1) * 512]
                                if dve_unit[kcp * 2 + sub]:
                                    # Schraudolph fast-exp on DVE: one
                                    # mult-add into int16 bits, bitcast bf16.
                                    nc.vector.tensor_scalar(
                                        out=dst.bitcast(I16),
                                        in0=stps[sub],
                                        scalar1=EXP_A,
                                        scalar2=EXP_B,
                                        op0=mybir.AluOpType.mult,
                                        op1=mybir.AluOpType.add,
                                    )
                                else:
                                    nc.scalar.activation(
                                        out=dst,
                                        in_=stps[sub],
                                        func=mybir.ActivationFunctionType.Exp,
                                    )
                        for half in range(2):
                            ov_q.append((ctx, kcp * 2 + half, ests, half))
                        while len(ov_q) > ov_delay:
                            do_ov(*ov_q.pop(0))
                        if pending_norm and (hp == 0 and kcp >= 5 or hp == 1):
                            emit_norm(*pending_norm.pop(0))
                        if hp == 1 and kcp % 2 == 1 and pending_outproj:
                            pqg, paot, pj = pending_outproj[0]
                            emit_outproj_j(pqg, paot, pj)
                            if pj == 3:
                                pending_outproj.pop(0)
                            else:
                                pending_outproj[0] = (pqg, paot, pj + 1)
            for item in ov_q:
                do_ov(*item)
            for item in pending_norm:
                emit_norm(*item)
            for pqg, paot, pj in pending_outproj:
                for j in range(pj, 4):
                    emit_outproj_j(pqg, paot, j)
    nc.finalize()
    return nc


_NC_CACHE = {}


def _get_nc(mode="v3"):
    if mode not in _NC_CACHE:
        _NC_CACHE[mode] = build_attention_kernel_v3()
    return _NC_CACHE[mode]


def shard_inputs(x, gamma, w_qkv, w_out, mode="v3"):
    """FULL inputs -> list of 8 per-core input maps.

    Host-side prep (fp64): gamma+1 and the 1/sqrt(dh) attention scale are
    folded into w_qkv; the per-token rms scale rs = sqrt(d)/||x_t|| is
    precomputed and shipped as a tiny [n] f32 tensor.
    """
    import ml_dtypes

    pdt = ml_dtypes.bfloat16
    d = x.shape[-1]
    dq = w_out.shape[0] // 4
    scale = DH ** -0.5
    gp1 = gamma.astype(np.float64) + 1.0
    w = w_qkv.astype(np.float64) * gp1[:, None]
    w[:, :d] *= scale  # q columns also absorb the softmax scale
    xs = x.astype(np.float64)
    rs = (d ** 0.5) / np.maximum(np.linalg.norm(xs, axis=-1), 1e-12)  # [b, n]
    xn = xs * rs[:, :, None]  # rms-normalized x (gamma fold lives in w)
    in_maps = []
    for c in range(NCORES):
        bi, g = c // 4, c % 4
        cs = slice(g * dq, (g + 1) * dq)
        wqkv_s = np.concatenate(
            [w[:, cs], w[:, d:][:, cs], w[:, 2 * d:][:, cs]], axis=1
        )
        xt = xn[bi].T.astype(pdt)  # [d, n]
        nhv = 4
        xt_tiled = np.ascontiguousarray(
            xt.reshape(d // P, P, nhv, x.shape[1] // nhv).transpose(2, 1, 0, 3)
        )
        wq = wqkv_s.astype(pdt)  # [d, 3*dq]
        wq_tiled = np.ascontiguousarray(
            wq.reshape(d // P, P, 3 * dq).transpose(1, 0, 2)
        )
        wo = w_out[cs, :].astype(pdt)  # [dq, d]
        wo_tiled = np.ascontiguousarray(
            wo.reshape(dq // P, P, d).transpose(1, 0, 2)
        )
        in_maps.append(
            {
                "xT": xt_tiled,
                "wqkv": wq_tiled,
                "wout": wo_tiled,
            }
        )
    return in_maps


def unshard_outputs(results):
    """8 partial [N, D] outputs -> full [B, N, D] (sum head groups per batch)."""
    outs = [np.asarray(r["out"], dtype=np.float32) for r in results]
    return np.stack(
        [
            outs[0] + outs[1] + outs[2] + outs[3],
            outs[4] + outs[5] + outs[6] + outs[7],
        ]
    ).astype(np.float32)


def run(x, gamma, w_qkv, w_out, mode="v3", **spmd_kwargs):
    nc = _get_nc(mode)
    in_maps = shard_inputs(x, gamma, w_qkv, w_out, mode)
    res = run_bass_kernel_spmd(nc, in_maps, list(range(NCORES)), **spmd_kwargs)
    return unshard_outputs(res.results), res


def kernel(x, gamma, w_qkv, w_out):
    out, _ = run(
        np.asarray(x), np.asarray(gamma), np.asarray(w_qkv), np.asarray(w_out)
    )
    return out
